# revision 33
# baseline (speedup 1.0000x reference)
"""ChurnLoss kernel for Trainium2, data-parallel over 8 NeuronCores.

Reference computation (see reference.py):
  inner:  sum over events j strictly inside each sequence (j in [s+1, e-2]) of
            -log(1 - p[j] + eps) + log(tau[j]) + (dt[j+1] + eps) / tau[j]
  term:   per sequence i, at j = offsets[i+1]-1:
            -log((1 - p[j]) * exp(-(t_to_now[i] + eps)/tau[j]) + p[j] + eps)
  loss = (inner + term) / n

Sharding: contiguous event ranges of S = n/8 events (1024 sequences) per core.
Each core computes a scalar partial; host sums the 8 partials (the "all-reduce")
and divides by n.

Fast path (uniform offsets, the layout produced by setup_inputs): each [128, F]
SBUF tile holds whole sequences per partition row, so "inner event" is a fixed
column pattern (col % 256 not in {0, 255}) expressible as a strided access
pattern -- no masks are needed.  The two logs run on the Scalar engine with
fused accum_out partial sums; the division term runs on the Vector engine as
reciprocal + multiply + reduce.  Terminal terms gather the per-sequence
last events with strided DMAs.  A general-offsets fallback (host-built mask +
host-gathered last indices, same device math) handles any other offsets.
"""

import numpy as np

import concourse.bacc as bacc
import concourse.mybir as mybir
from concourse import bass_isa, tile
from concourse.bass_utils import run_bass_kernel_spmd

# Steer Bacc's activation-table pass to the combined natural_log+exp set so Ln
# and Exp ops share one table load (set ids stay aligned with act_info.json;
# we only hide Ln/Exp from the earlier single-function sets so the fixpoint
# pass lands on "natural_log_exp_and_others", whose tables are at least as
# precise: ln 400 buckets vs natural_log's 40).
_orig_get_tables = bacc.get_activation_tables


def _patched_get_tables(arch):
    tables = {k: set(v) for k, v in _orig_get_tables(arch).items()}
    if "natural_log_exp_and_others" in tables:
        tables.get("natural_log", set()).discard(
            mybir.ActivationFunctionType.Ln)
        tables.get("exp_and_others", set()).discard(
            mybir.ActivationFunctionType.Exp)
    return tables


bacc.get_activation_tables = _patched_get_tables

F32 = mybir.dt.float32
AF = mybir.ActivationFunctionType
ALU = mybir.AluOpType
AX = mybir.AxisListType

TOTAL = 2097152
B = 8192
L = 256
NCORES = 8
S = TOTAL // NCORES          # events per core
SEQS = B // NCORES           # sequences per core
P = 128
NCHUNK = 4
F = S // (P * NCHUNK)        # free-dim columns per chunk tile
BLK = F // L                 # sequences per partition row per chunk
ROWSEQ = (S // P) // L       # sequences per partition row over the whole shard
EPS = 1e-5


def _make_bias(nc, pool, tag, val):
    t = pool.tile([P, 1], F32, tag=tag)
    nc.vector.memset(t[:], val)
    return t


def _emit_terminal(nc, pool, pl, tl, tn, ceps, sacc_col):
    """From [128, SEQS/128] tiles pl (p_last), tl (tau_last), tn (t_to_now),
    accumulate sum(log((1-p)*exp(-(t+eps)/tau) + p + eps)) into sacc_col."""
    shape = [P, SEQS // P]
    rl = pool.tile(shape, F32, tag="rl")
    nc.vector.reciprocal(rl[:], tl[:])
    xe = pool.tile(shape, F32, tag="xe")
    nc.vector.tensor_scalar_add(xe[:], tn[:], EPS)
    x = pool.tile(shape, F32, tag="x")
    nc.vector.tensor_tensor(x[:], xe[:], rl[:], ALU.mult)
    sv = pool.tile(shape, F32, tag="sv")
    nc.scalar.activation(sv[:], x[:], AF.Exp, scale=-1.0)
    v = pool.tile(shape, F32, tag="v")
    nc.vector.tensor_scalar(v[:], pl[:], -1.0, 1.0, ALU.mult, ALU.add)
    w = pool.tile(shape, F32, tag="w")
    nc.vector.tensor_tensor(w[:], v[:], sv[:], ALU.mult)
    y = pool.tile(shape, F32, tag="y")
    nc.vector.tensor_tensor(y[:], w[:], pl[:], ALU.add)
    z = pool.tile(shape, F32, tag="z")
    nc.scalar.activation(z[:], y[:], AF.Ln, bias=ceps[:], accum_out=sacc_col)


FAST_CFG = dict(chunks=(4, 4), scratch_bufs=4, fuse_log=False,
                dtype16=True, qsum="dve")
F16 = mybir.dt.float16

# Packed-layout fast path: one fp16 input tensor per core holding
# [chunk0: 128 x (3f+24) | chunk1..: 128 x 3f] where each partition row of a
# chunk is [tau row | dt1 row | p row], and chunk0 rows carry a 24-column tail
# [p_last(8) | tau_last(8) | t_to_now(8)].  One dma_start per chunk -- the
# dominant cost on this runtime is ~1.8us of fixed overhead per dma_start.
PACKED_CHUNKS = (2, 2, 2, 2)
TAIL = 3 * (SEQS // P)       # 24 tail columns on chunk 0


def _packed_len():
    return 3 * S + P * TAIL


def _build_packed(chunks=PACKED_CHUNKS, scratch_bufs=4, hw_loop_iters=1,
                  dma_only=False, pin_bufs=None, skip_terminal=False,
                  skip_finals=False, raw_out=True):
    nchunk = len(chunks)
    assert sum(chunks) == (S // P) // L
    nc = bacc.Bacc("TRN2", target_bir_lowering=False, debug=False,
                   num_devices=NCORES)
    ev_d = nc.declare_dram_parameter("ev_s", [_packed_len()], F16,
                                     isOutput=False)
    out_w = (3 * nchunk + 1) if raw_out else 1
    out_d = nc.declare_dram_parameter("partial", [P, out_w], F32,
                                     isOutput=True)
    G = SEQS // P   # 8 terminal values per partition row

    with tile.TileContext(nc) as tc:
        with (
            tc.tile_pool(name="pin", bufs=pin_bufs or nchunk) as pin,
            tc.tile_pool(name="scratch", bufs=scratch_bufs) as scratch,
            tc.tile_pool(name="small", bufs=1) as small,
        ):
            acc = small.tile([P, 3 * nchunk + 1], F32, tag="acc")
            sacc = acc[:, 0:2 * nchunk + 1]
            vacc = acc[:, 2 * nchunk + 1:3 * nchunk + 1]
            c1e = _make_bias(nc, small, "c1e", 1.0 + EPS)
            ceps = _make_bias(nc, small, "ceps", EPS)

            def emit_body():
                big = []
                off = 0
                for k, blk in enumerate(chunks):
                    f = blk * L
                    width = 3 * f + (TAIL if k == 0 else 0)
                    t = pin.tile([P, width], F16, tag=f"big{k}")
                    nc.sync.dma_start(
                        t[:], ev_d[off:off + P * width].rearrange(
                            "(p f) -> p f", p=P))
                    big.append(t)
                    off += P * width
                if dma_only:
                    nc.sync.dma_start(out_d[:, 0:1], c1e[:, :])
                    return

                for k, blk in enumerate(chunks):
                    f = blk * L
                    t = big[k]
                    tau_v, dt1_v, p_v = (t[:, 0:f], t[:, f:2 * f],
                                         t[:, 2 * f:3 * f])

                    def inner3(ap):
                        return ap.rearrange("p (b l) -> p b l",
                                            l=L)[:, :, 1:L - 1]

                    w = blk * (L - 2)

                    def compact3(s):
                        return s[:].rearrange("p (b l) -> p b l", l=L - 2)

                    r = scratch.tile([P, w], F16, tag="r")
                    with nc.allow_low_precision("fp16 bandwidth mode"):
                        nc.vector.reciprocal(compact3(r), inner3(tau_v))
                    q = scratch.tile([P, w], F16, tag="q")
                    nc.vector.tensor_tensor(compact3(q), inner3(dt1_v),
                                            compact3(r), ALU.mult)
                    qj = scratch.tile([P, w], F16, tag="qj")
                    nc.vector.tensor_scalar(qj[:], q[:], 1.0, None, ALU.mult,
                                            ALU.add,
                                            accum_out=vacc[:, k:k + 1])
                    t2 = scratch.tile([P, w], F16, tag="t2")
                    nc.scalar.activation(compact3(t2), inner3(tau_v), AF.Ln,
                                         accum_out=sacc[:, k:k + 1])
                    t1 = scratch.tile([P, w], F16, tag="t1")
                    nc.scalar.activation(
                        compact3(t1), inner3(p_v), AF.Ln,
                        bias=c1e[:], scale=-1.0,
                        accum_out=sacc[:, nchunk + k:nchunk + k + 1])

                # terminal columns live in chunk0's tail
                if not skip_terminal:
                    f0 = chunks[0] * L
                    pl = big[0][:, 3 * f0:3 * f0 + G]
                    tl = big[0][:, 3 * f0 + G:3 * f0 + 2 * G]
                    tnv = big[0][:, 3 * f0 + 2 * G:3 * f0 + 3 * G]
                    plf = small.tile([P, G], F32, tag="plf")
                    nc.vector.tensor_copy(plf[:], pl)
                    tn = small.tile([P, G], F32, tag="tn")
                    nc.vector.tensor_copy(tn[:], tnv)
                    _emit_terminal(nc, small, plf, tl, tn, ceps,
                                   sacc[:, 2 * nchunk:2 * nchunk + 1])
                else:
                    nc.vector.memset(sacc[:, 2 * nchunk:2 * nchunk + 1], 0.0)
                if skip_finals:
                    nc.sync.dma_start(out_d[:, 0:1], c1e[:, :])
                    return

                if raw_out:
                    nc.sync.dma_start(out_d[:, :], acc[:, :])
                else:
                    ps = small.tile([P, 1], F32, tag="ps")
                    nc.vector.reduce_sum(ps[:], sacc[:, 0:nchunk], axis=AX.X)
                    ns = small.tile([P, 1], F32, tag="ns")
                    nc.vector.reduce_sum(ns[:], sacc[:, nchunk:2 * nchunk + 1],
                                         axis=AX.X)
                    pv = small.tile([P, 1], F32, tag="pv")
                    nc.vector.reduce_sum(pv[:], vacc[:, :], axis=AX.X)
                    d = small.tile([P, 1], F32, tag="d")
                    nc.vector.tensor_tensor(d[:], ps[:], ns[:], ALU.subtract)
                    pt = small.tile([P, 1], F32, tag="pt")
                    nc.vector.tensor_tensor(pt[:], d[:], pv[:], ALU.add)
                    nc.sync.dma_start(out_d[:, :], pt[:, :])

            if hw_loop_iters > 1:
                with tc.For_i(0, hw_loop_iters, 1):
                    emit_body()
            else:
                emit_body()
    nc.finalize()
    return nc


def pack_inputs(p16, tau16, dt116, tnow, chunks=PACKED_CHUNKS):
    """Build the packed per-core fp16 arrays.  p16/tau16/dt116: [TOTAL] fp16,
    tnow: [B] float -> fp16 tail."""
    G = SEQS // P
    pl = p16[L - 1::L].reshape(NCORES, P, G)
    tl = tau16[L - 1::L].reshape(NCORES, P, G)
    tn = tnow.astype(np.float16).reshape(NCORES, P, G)
    tail = np.concatenate([pl, tl, tn], axis=2)          # [NC, P, TAIL]
    out = []
    for c in range(NCORES):
        parts = []
        off = c * S
        for k, blk in enumerate(chunks):
            f = blk * L
            n_el = P * f
            tau_c = tau16[off:off + n_el].reshape(P, f)
            dt_c = dt116[off:off + n_el].reshape(P, f)
            p_c = p16[off:off + n_el].reshape(P, f)
            cols = [tau_c, dt_c, p_c]
            if k == 0:
                cols.append(tail[c])
            parts.append(np.concatenate(cols, axis=1).reshape(-1))
            off += n_el
        out.append(np.concatenate(parts))
    return out


def _build_fast(chunks=(2, 2, 2, 2), scratch_bufs=4, fuse_log=False,
                dtype16=False, qsum="dve", terminal_first=False, reps=1,
                hw_loop_iters=1, bench_stages=None):
    """Uniform-offsets program: inner columns selected by strided APs.

    chunks: per-chunk width in 256-column blocks per partition row (sums to
    (S/128)/256 = 8).  fuse_log: compute log((1-p+eps)/tau) as one ACT log of
    (1-p+eps)*r instead of two separate logs.  dtype16: event arrays arrive as
    float16 (host-converted), halving the HBM traffic; the q=dt/tau path runs
    in fp16 (the +eps there is below fp16 ulp and is dropped -- ~1e-5 relative
    effect), logs and all accumulation stay fp32.  Output is the per-partition
    [128, 1] partial vector; the host sums partitions and cores.
    """
    nchunk = len(chunks)
    assert sum(chunks) == (S // P) // L
    DT = F16 if dtype16 else F32
    nc = bacc.Bacc("TRN2", target_bir_lowering=False, debug=False,
                   num_devices=NCORES)
    p_d = nc.declare_dram_parameter("p_s", [S], DT, isOutput=False)
    tau_d = nc.declare_dram_parameter("tau_s", [S], DT, isOutput=False)
    dt1_d = nc.declare_dram_parameter("dt1_s", [S], DT, isOutput=False)
    tn_d = nc.declare_dram_parameter("tnow_s", [SEQS], F32, isOutput=False)
    out_d = nc.declare_dram_parameter("partial", [P, 1], F32, isOutput=True)

    with tile.TileContext(nc) as tc:
        with (
            tc.tile_pool(name="pin", bufs=nchunk) as pin,
            tc.tile_pool(name="scratch", bufs=scratch_bufs) as scratch,
            tc.tile_pool(name="small", bufs=1) as small,
        ):
            # Per-sequence last-event gathers: element (row, seq, 255) of the
            # shard viewed as [128, ROWSEQ, 256].  Issued on the GPSIMD SWDGE
            # queues so the SP HWDGE queue streams the big chunk loads from
            # cycle zero.
            pl = small.tile([P, SEQS // P], DT, tag="pl")
            tl = small.tile([P, SEQS // P], DT, tag="tl")
            tn = small.tile([P, SEQS // P], F32, tag="tn")
            p_seq = p_d[:].rearrange("(a b c) -> a b c", a=P, c=L)
            tau_seq = tau_d[:].rearrange("(a b c) -> a b c", a=P, c=L)
            nc.gpsimd.dma_start(
                pl[:].rearrange("p (b o) -> p b o", o=1), p_seq[:, :, L - 1:L])
            nc.gpsimd.dma_start(
                tl[:].rearrange("p (b o) -> p b o", o=1), tau_seq[:, :, L - 1:L])
            nc.gpsimd.dma_start(tn[:], tn_d[:].rearrange("(a b) -> a b", a=P))
            if dtype16:
                plf = small.tile([P, SEQS // P], F32, tag="plf")
                nc.vector.tensor_copy(plf[:], pl[:])
            else:
                plf = pl

            # sacc columns (all negative contributions when fuse_log):
            #   fused: 0..n-1 = sum log((1-p+eps)/tau), n = terminal log sum.
            #   else:  0..n-1 = sum log tau [+], n..2n-1 = sum log(1-p+eps) [-],
            #          2n = terminal log sum [-].
            sacc_w = (nchunk + 1) if fuse_log else (2 * nchunk + 1)
            sacc = small.tile([P, sacc_w], F32, tag="sacc")
            vacc = small.tile([P, nchunk], F32, tag="vacc")
            c1e = _make_bias(nc, small, "c1e", 1.0 + EPS)
            ceps = _make_bias(nc, small, "ceps", EPS)

            flat_p = p_d[:]
            flat_tau = tau_d[:]
            flat_dt1 = dt1_d[:]

            def emit_body_bench():
                off = 0
                for k, blk in enumerate(chunks):
                    f = blk * L
                    n_el = P * f
                    tau_t = pin.tile([P, f], DT, tag="tau")
                    dt1_t = pin.tile([P, f], DT, tag="dt1")
                    p_t = pin.tile([P, f], DT, tag="p")
                    nc.sync.dma_start(
                        tau_t[:],
                        flat_tau[off:off + n_el].rearrange("(p f) -> p f", p=P))
                    nc.sync.dma_start(
                        dt1_t[:],
                        flat_dt1[off:off + n_el].rearrange("(p f) -> p f", p=P))
                    nc.sync.dma_start(
                        p_t[:],
                        flat_p[off:off + n_el].rearrange("(p f) -> p f", p=P))
                    inner = lambda ap: ap.rearrange(
                        "p (b l) -> p b l", l=L)[:, :, 1:L - 1]
                    w = blk * (L - 2)
                    comp = lambda t: t[:].rearrange("p (b l) -> p b l", l=L - 2)
                    if "dve" in bench_stages:
                        r = scratch.tile([P, w], DT, tag="r")
                        with nc.allow_low_precision("bench"):
                            nc.vector.reciprocal(comp(r), inner(tau_t[:]))
                        q = scratch.tile([P, w], DT, tag="q")
                        nc.vector.tensor_tensor(comp(q), inner(dt1_t[:]),
                                                comp(r), ALU.mult)
                        qj = scratch.tile([P, w], DT, tag="qj")
                        nc.vector.tensor_scalar(qj[:], q[:], 1.0, None,
                                                ALU.mult, ALU.add,
                                                accum_out=vacc[:, k:k + 1])
                    if "act" in bench_stages:
                        t2 = scratch.tile([P, w], DT, tag="t2")
                        nc.scalar.activation(comp(t2), inner(tau_t[:]), AF.Ln,
                                             accum_out=sacc[:, k:k + 1])
                        t1 = scratch.tile([P, w], DT, tag="t1")
                        nc.scalar.activation(
                            comp(t1), inner(p_t[:]), AF.Ln,
                            bias=c1e[:], scale=-1.0,
                            accum_out=sacc[:, nchunk + k:nchunk + k + 1])
                    off += n_el
                nc.sync.dma_start(out_d[:, :], c1e[:, :])

            def emit_body():
              if bench_stages is not None:
                emit_body_bench()
                return
              if terminal_first:
                _emit_terminal(nc, small, plf, tl, tn, ceps,
                               sacc[:, sacc_w - 1:sacc_w])

              off = 0
              for k, blk in enumerate(chunks):
                f = blk * L
                n_el = P * f

                def chunk_view(flat):
                    return flat[off:off + n_el].rearrange("(p f) -> p f", p=P)

                # tau first (feeds both DVE reciprocal and the log path), then
                # dt1 (GPSIMD +eps), then p -- shortens the tail on the last
                # chunk.
                tau_t = pin.tile([P, f], DT, tag="tau")
                dt1_t = pin.tile([P, f], DT, tag="dt1")
                p_t = pin.tile([P, f], DT, tag="p")
                nc.sync.dma_start(tau_t[:], chunk_view(flat_tau))
                nc.sync.dma_start(dt1_t[:], chunk_view(flat_dt1))
                nc.sync.dma_start(p_t[:], chunk_view(flat_p))

                def inner3(ap):  # [128, f] -> inner columns [128, blk, L-2]
                    return ap.rearrange("p (b l) -> p b l", l=L)[:, :, 1:L - 1]

                w = blk * (L - 2)

                def compact3(t):  # [128, w] scratch -> [128, blk, L-2]
                    return t[:].rearrange("p (b l) -> p b l", l=L - 2)

                r = scratch.tile([P, w], DT, tag="r")
                with nc.allow_low_precision("fp16 bandwidth mode"):
                    nc.vector.reciprocal(compact3(r), inner3(tau_t[:]))
                q = scratch.tile([P, w], DT, tag="q")
                if dtype16:
                    # +eps is below fp16 ulp; q = dt1 * (1/tau) directly, then
                    # sum via a fused-accumulate pass on DVE or ACT.
                    nc.vector.tensor_tensor(compact3(q), inner3(dt1_t[:]),
                                            compact3(r), ALU.mult)
                    qj = scratch.tile([P, w], DT, tag="qj")
                    if qsum == "act":
                        nc.scalar.activation(qj[:], q[:], AF.Copy,
                                             accum_out=vacc[:, k:k + 1])
                    else:
                        nc.vector.tensor_scalar(qj[:], q[:], 1.0, None,
                                                ALU.mult, ALU.add,
                                                accum_out=vacc[:, k:k + 1])
                else:
                    dte = scratch.tile([P, w], F32, tag="dte")
                    nc.gpsimd.tensor_scalar_add(compact3(dte),
                                                inner3(dt1_t[:]), EPS)
                    nc.vector.tensor_tensor(q[:], dte[:], r[:], ALU.mult)
                    nc.vector.reduce_sum(vacc[:, k:k + 1], q[:], axis=AX.X)
                if fuse_log:
                    # a = 1 - p + eps (GPSIMD), u = a * r (DVE),
                    # log(u) = log(1-p+eps) - log(tau)  [negative contribution]
                    a = scratch.tile([P, w], DT, tag="a")
                    nc.gpsimd.tensor_scalar(compact3(a), inner3(p_t[:]),
                                            -1.0, 1.0 + EPS,
                                            ALU.mult, ALU.add)
                    u = scratch.tile([P, w], DT, tag="u")
                    nc.vector.tensor_tensor(u[:], a[:], r[:], ALU.mult)
                    t12 = scratch.tile([P, w], DT, tag="t12")
                    nc.scalar.activation(t12[:], u[:], AF.Ln,
                                         accum_out=sacc[:, k:k + 1])
                else:
                    t2 = scratch.tile([P, w], DT, tag="t2")
                    nc.scalar.activation(compact3(t2), inner3(tau_t[:]), AF.Ln,
                                         accum_out=sacc[:, k:k + 1])
                    t1 = scratch.tile([P, w], DT, tag="t1")
                    nc.scalar.activation(
                        compact3(t1), inner3(p_t[:]), AF.Ln,
                        bias=c1e[:], scale=-1.0,
                        accum_out=sacc[:, nchunk + k:nchunk + k + 1])
                off += n_el

              if not terminal_first:
                _emit_terminal(nc, small, plf, tl, tn, ceps,
                               sacc[:, sacc_w - 1:sacc_w])

              pv = small.tile([P, 1], F32, tag="pv")
              nc.vector.reduce_sum(pv[:], vacc[:], axis=AX.X)
              if fuse_log:
                # partial = sum(q) - sum(log((1-p+eps)/tau)) - sum(term)
                ns = small.tile([P, 1], F32, tag="ns")
                nc.vector.reduce_sum(ns[:], sacc[:], axis=AX.X)
                pt = small.tile([P, 1], F32, tag="pt")
                nc.vector.tensor_tensor(pt[:], pv[:], ns[:], ALU.subtract)
              else:
                ps = small.tile([P, 1], F32, tag="ps")
                nc.vector.reduce_sum(ps[:], sacc[:, 0:nchunk], axis=AX.X)
                ns = small.tile([P, 1], F32, tag="ns")
                nc.vector.reduce_sum(ns[:], sacc[:, nchunk:sacc_w], axis=AX.X)
                d = small.tile([P, 1], F32, tag="d")
                nc.vector.tensor_tensor(d[:], ps[:], ns[:], ALU.subtract)
                pt = small.tile([P, 1], F32, tag="pt")
                nc.vector.tensor_tensor(pt[:], d[:], pv[:], ALU.add)
              nc.sync.dma_start(out_d[:, :], pt[:, :])

            if hw_loop_iters > 1:
                with tc.For_i(0, hw_loop_iters, 1):
                    for _rep in range(reps):
                        emit_body()
            else:
                for _rep in range(reps):
                    emit_body()
    nc.finalize()
    return nc


def _build_general():
    """Arbitrary-offsets program: host supplies a 0/1 inner mask per event and
    pre-gathered per-sequence last-event values."""
    nc = bacc.Bacc("TRN2", target_bir_lowering=False, debug=False,
                   num_devices=NCORES)
    p_d = nc.declare_dram_parameter("p_s", [S], F32, isOutput=False)
    tau_d = nc.declare_dram_parameter("tau_s", [S], F32, isOutput=False)
    dt1_d = nc.declare_dram_parameter("dt1_s", [S], F32, isOutput=False)
    m_d = nc.declare_dram_parameter("m_s", [S], F32, isOutput=False)
    pl_d = nc.declare_dram_parameter("pl_s", [SEQS], F32, isOutput=False)
    tl_d = nc.declare_dram_parameter("tl_s", [SEQS], F32, isOutput=False)
    tn_d = nc.declare_dram_parameter("tnow_s", [SEQS], F32, isOutput=False)
    out_d = nc.declare_dram_parameter("partial", [1, 1], F32, isOutput=True)

    with tile.TileContext(nc) as tc:
        with (
            tc.tile_pool(name="pin", bufs=NCHUNK) as pin,
            tc.tile_pool(name="scratch", bufs=2) as scratch,
            tc.tile_pool(name="small", bufs=1) as small,
        ):
            pl = small.tile([P, SEQS // P], F32, tag="pl")
            tl = small.tile([P, SEQS // P], F32, tag="tl")
            tn = small.tile([P, SEQS // P], F32, tag="tn")
            nc.sync.dma_start(pl[:], pl_d[:].rearrange("(a b) -> a b", a=P))
            nc.sync.dma_start(tl[:], tl_d[:].rearrange("(a b) -> a b", a=P))
            nc.sync.dma_start(tn[:], tn_d[:].rearrange("(a b) -> a b", a=P))

            # macc columns: 0..N-1 sum(m*log tau) [+], N..2N-1 sum(m*q) [+],
            # 2N..3N-1 sum(m*log(1-p+eps)) [-].  sacc: terminal log sum [-].
            macc = small.tile([P, 3 * NCHUNK], F32, tag="macc")
            sacc = small.tile([P, 1], F32, tag="sacc")
            c1e = _make_bias(nc, small, "c1e", 1.0 + EPS)
            ceps = _make_bias(nc, small, "ceps", EPS)

            chv = lambda d: d[:].rearrange("(k p f) -> k p f", p=P, f=F)
            p_ch, tau_ch, dt1_ch, m_ch = map(chv, (p_d, tau_d, dt1_d, m_d))

            for k in range(NCHUNK):
                p_t = pin.tile([P, F], F32, tag="p")
                tau_t = pin.tile([P, F], F32, tag="tau")
                dt1_t = pin.tile([P, F], F32, tag="dt1")
                m_t = pin.tile([P, F], F32, tag="m")
                nc.sync.dma_start(p_t[:], p_ch[k, :, :])
                nc.sync.dma_start(tau_t[:], tau_ch[k, :, :])
                nc.sync.dma_start(dt1_t[:], dt1_ch[k, :, :])
                nc.sync.dma_start(m_t[:], m_ch[k, :, :])

                t1 = scratch.tile([P, F], F32, tag="t1")
                nc.scalar.activation(t1[:], p_t[:], AF.Ln,
                                     bias=c1e[:], scale=-1.0)
                t2 = scratch.tile([P, F], F32, tag="t2")
                nc.scalar.activation(t2[:], tau_t[:], AF.Ln)
                r = scratch.tile([P, F], F32, tag="r")
                nc.vector.reciprocal(r[:], tau_t[:])
                dte = scratch.tile([P, F], F32, tag="dte")
                nc.scalar.activation(dte[:], dt1_t[:], AF.Identity,
                                     bias=ceps[:])
                q = scratch.tile([P, F], F32, tag="q")
                nc.vector.tensor_tensor(q[:], dte[:], r[:], ALU.mult)
                j1 = scratch.tile([P, F], F32, tag="j1")
                nc.vector.tensor_tensor(j1[:], t2[:], m_t[:], ALU.mult)
                nc.vector.reduce_sum(macc[:, k:k + 1], j1[:], axis=AX.X)
                j2 = scratch.tile([P, F], F32, tag="j2")
                nc.vector.tensor_tensor(j2[:], q[:], m_t[:], ALU.mult)
                nc.vector.reduce_sum(macc[:, NCHUNK + k:NCHUNK + k + 1],
                                     j2[:], axis=AX.X)
                j3 = scratch.tile([P, F], F32, tag="j3")
                nc.vector.tensor_tensor(j3[:], t1[:], m_t[:], ALU.mult)
                nc.vector.reduce_sum(macc[:, 2 * NCHUNK + k:2 * NCHUNK + k + 1],
                                     j3[:], axis=AX.X)

            _emit_terminal(nc, small, pl, tl, tn, ceps, sacc[:, 0:1])

            pm = small.tile([P, 1], F32, tag="pm")
            nc.vector.reduce_sum(pm[:], macc[:, 0:2 * NCHUNK], axis=AX.X)
            nm = small.tile([P, 1], F32, tag="nm")
            nc.vector.reduce_sum(nm[:], macc[:, 2 * NCHUNK:3 * NCHUNK], axis=AX.X)
            d = small.tile([P, 1], F32, tag="d")
            nc.vector.tensor_tensor(d[:], pm[:], nm[:], ALU.subtract)
            pt = small.tile([P, 1], F32, tag="pt")
            nc.vector.tensor_tensor(pt[:], d[:], sacc[:], ALU.subtract)
            ar = small.tile([P, 1], F32, tag="ar")
            nc.gpsimd.partition_all_reduce(ar[:], pt[:], channels=P,
                                           reduce_op=bass_isa.ReduceOp.add)
            nc.sync.dma_start(out_d[:, :], ar[0:1, 0:1])
    nc.finalize()
    return nc


_CACHE = {}


def _get_nc(which):
    if which not in _CACHE:
        _CACHE[which] = (_build_packed() if which == "fast"
                         else _build_general())
    return _CACHE[which]


def _shift_dt(dt):
    dt1 = np.empty_like(dt)
    dt1[:-1] = dt[1:]
    dt1[-1] = 0.0
    return dt1


def _inner_mask(offsets):
    """Exact replica of the reference's searchsorted-based inner mask."""
    idx = np.arange(TOTAL)
    seg = np.searchsorted(offsets, idx, side="right") - 1
    starts = offsets[seg]
    ends = offsets[np.minimum(seg + 1, offsets.shape[0] - 1)]
    return ((idx > starts) & (idx < ends - 1)).astype(np.float32)


def kernel(dt, tau, p, t_to_now, offsets):
    dt = np.ascontiguousarray(dt, dtype=np.float32)
    tau = np.ascontiguousarray(tau, dtype=np.float32)
    p = np.ascontiguousarray(p, dtype=np.float32)
    t_to_now = np.ascontiguousarray(t_to_now, dtype=np.float32)
    offsets = np.asarray(offsets)

    uniform = offsets.shape == (B + 1,) and np.array_equal(
        offsets, np.arange(0, TOTAL + 1, L, dtype=offsets.dtype))

    dt1 = _shift_dt(dt)
    in_maps = []
    if uniform:
        nc = _get_nc("fast")
        evs = pack_inputs(p.astype(np.float16), tau.astype(np.float16),
                          dt1.astype(np.float16), t_to_now)
        in_maps = [{"ev_s": ev} for ev in evs]
    else:
        nc = _get_nc("general")
        m = _inner_mask(offsets)
        last_idx = offsets[1:].astype(np.int64) - 1
        pl = p[last_idx]
        tl = tau[last_idx]
        for c in range(NCORES):
            lo, hi = c * S, (c + 1) * S
            sl, sh = c * SEQS, (c + 1) * SEQS
            in_maps.append({
                "p_s": p[lo:hi],
                "tau_s": tau[lo:hi],
                "dt1_s": dt1[lo:hi],
                "m_s": m[lo:hi],
                "pl_s": np.ascontiguousarray(pl[sl:sh]),
                "tl_s": np.ascontiguousarray(tl[sl:sh]),
                "tnow_s": t_to_now[sl:sh],
            })

    global LAST_RESULTS
    res = run_bass_kernel_spmd(nc, in_maps, core_ids=list(range(NCORES)),
                               trace=TRACE, **(RUN_KWARGS or {}))
    LAST_RESULTS = res
    total = np.float64(0.0)
    for c in range(NCORES):
        arr = res.results[c]["partial"].astype(np.float64)
        if uniform:
            # raw accumulator columns: [log tau (n) | log(1-p+eps) (n) |
            #  terminal (1) | q (n)]; middle group and terminal are negative.
            n = len(PACKED_CHUNKS)
            total += (arr[:, 0:n].sum() - arr[:, n:2 * n + 1].sum()
                      + arr[:, 2 * n + 1:3 * n + 1].sum())
        else:
            total += arr.sum()
    return np.float32(total / TOTAL)


# Profiling knobs for local testing (test.py sets TRACE=True to capture an
# NTFF trace); the graded path leaves these defaults untouched.
TRACE = False
RUN_KWARGS = None
LAST_RESULTS = None
